# revision 1
# baseline (speedup 1.0000x reference)
"""Masked attention head (BATCH=8, SEQ=2048, HEAD_DIM=128) on 8 trn2 cores.

Math per batch b (L = event_lengths[b]): scores = q @ k^T / sqrt(128), rows
and cols >= L masked to -1e9, softmax, @ v.  Rows >= L therefore get the
uniform average mean(v); valid rows attend only to cols < L.

Device strategy (primary, "packed"): the host knows L at call time, so only
the valid work is computed.  A work unit is (1024-wide qi-block x 128-deep
ki-chunk); the global unit stream is balance-cut across the 8 cores and each
core gets its OWN specialized Bass program (no SPMD padding).  Per unit:
S^T = kT_chunk^T @ qT_block via a single-pass f32r matmul (ki on partitions,
qi free), exp on the scalar engine with the ki-mask fused as a per-partition
-1e9 bias, AV accumulated in PSUM as out^T = sum v_chunk^T P^T_chunk, and
bf16 softmax-denominator partials accumulated on the vector engine.  The
host sums segment partials, normalizes, and blends invalid rows with
mean(v).  Programs are compiled per call and dispatched concurrently to the
8 cores through the same bass2jax/PJRT path run_bass_kernel_spmd uses.

Fallback (on any failure): one dense SPMD program on all 8 cores (batch b ->
core b), full 2048x2048 masked attention, run via run_bass_kernel_spmd.
"""

import numpy as np
from concurrent.futures import ThreadPoolExecutor

import jax

import concourse.bass as bass
import concourse.mybir as mybir
import concourse.tile as tile
from concourse import bass2jax
from concourse.bass_utils import run_bass_kernel_spmd
from concourse.vector_clock import ScopedClock

try:
    import ml_dtypes
    _BF16 = np.dtype(ml_dtypes.bfloat16)
except ImportError:  # pragma: no cover
    _BF16 = np.float32

BATCH, SEQ, D = 8, 2048, 128
NCH = SEQ // 128
BQ = 1024
SCALE = 1.0 / np.sqrt(D)
NEG = -1.0e9

f32 = mybir.dt.float32
f32r = mybir.dt.float32r
bf16 = mybir.dt.bfloat16

# ---------------------------------------------------------------------------
# Workaround for this walrus build: at most ONE sync-wait command per
# instruction.  Tile attaches one wait per depended-on logical processor;
# hoist extra waits onto dedicated single-wait nops on the same engine queue
# (queues execute serially, so this is semantics-preserving).
# ---------------------------------------------------------------------------
_nop_counter = [0]


def _fresh_nop(engine, wait):
    _nop_counter[0] += 1
    n = mybir.InstNoOp(name=f"waitnop-{_nop_counter[0]}", ins=[], outs=[])
    n.engine = engine
    n.sync_info = mybir.SyncInfo(on_wait=[wait], on_update=[])
    return n


def _split_multi_waits(nc):
    for f in nc.m.functions:
        for bb in f.blocks:
            insts = bb.instructions
            out = []
            changed = False
            for inst in insts:
                si = inst.sync_info
                waits = list(si.on_wait) if si else []
                if len(waits) > 1:
                    for w in waits[:-1]:
                        out.append(_fresh_nop(inst.engine, w))
                    inst.sync_info = mybir.SyncInfo(
                        on_wait=[waits[-1]], on_update=list(si.on_update)
                    )
                    changed = True
                out.append(inst)
            if changed:
                insts.clear()
                insts.extend(out)


def _drain_and_barrier_split(self, tick_clock, wait_clock):
    drain_inst = self.nc.sync.drain()
    wait_clock.add_sem_waits(
        drain_inst.ins, ScopedClock({None: tick_clock.global_clock})
    )
    self.nc.all_engine_barrier()
    assert self.sems is not None
    popped = self.nc._tile_sem_poison_stack.pop()
    assert popped is self._sem_poison
    self.nc.clear_and_free_semaphores(list(self.sems.allocated().values()))
    self.nc.all_engine_barrier()


tile.TileContext._drain_and_barrier = _drain_and_barrier_split


# ---------------------------------------------------------------------------
# Packed path: planning
# ---------------------------------------------------------------------------
def _plan(lens):
    """Per-core segment lists [(b, qblk, k0, nk, w), ...]: LPT over
    (batch, qi-block) blocks with splitting, sliver guard, and a
    per-segment overhead term.  w is the unit qi-width: 512 when the
    block's valid qi-extent fits in 512, else 1024."""
    OVH = 0.3
    MIN_PIECE = 3
    W_COST = {256: 0.38, 512: 0.57, 768: 0.80, 1024: 1.0}
    blocks = []
    for b in range(BATCH):
        L = int(lens[b])
        if L <= 0:
            continue
        nK = -(-L // 128)
        nQ = -(-L // BQ)
        for qblk in range(nQ):
            extent = min(BQ, L - qblk * BQ)
            w = min(BQ, 256 * -(-extent // 256))
            blocks.append((nK * W_COST[w], nK, b, qblk, w))
    if not blocks:
        return [[] for _ in range(BATCH)]
    total_eff = sum(cost + OVH for (cost, _, _, _, _) in blocks)

    def plan_with(target):
        loads = [0.0] * BATCH
        cores = [[] for _ in range(BATCH)]
        pieces = [(cost, nK, b, qblk, w, 0)
                  for (cost, nK, b, qblk, w) in blocks]
        while pieces:
            cost, nk, b, qblk, w, k0 = pieces.pop(0)
            wc = W_COST[w]
            c = min(range(BATCH), key=lambda i: loads[i])
            room = (target - loads[c] - OVH) / wc
            if nk > room + MIN_PIECE and nk > 2 * MIN_PIECE:
                take = int(max(MIN_PIECE, min(nk - MIN_PIECE, round(room))))
                cores[c].append((b, qblk, k0, take, w))
                loads[c] += take * wc + OVH
                rest = nk - take
                rcost = rest * wc
                idx = 0
                while idx < len(pieces) and pieces[idx][0] > rcost:
                    idx += 1
                pieces.insert(idx, (rcost, rest, b, qblk, w, k0 + take))
            else:
                cores[c].append((b, qblk, k0, nk, w))
                loads[c] += nk * wc + OVH
        return cores, max(loads)

    W_TIME = {256: 0.40, 512: 0.58, 768: 0.81, 1024: 1.0}

    def real_score(cores):
        def core_time(segs):
            t = sum(nk * W_TIME[w] for (_, _, _, nk, w) in segs)
            # small pieces cost disproportionately (export churn)
            t += sum(0.05 if nk >= 5 else 0.35 for (_, _, _, nk, _) in segs)
            return t
        return max((core_time(s) for s in cores), default=0.0) \
            if any(cores) else 0.0

    def refine(cores):
        refined = []
        for segs in cores:
            out = []
            for (b, qblk, k0, nk, w) in sorted(
                    segs, key=lambda s: -s[3] * s[4]):
                if nk >= 9:
                    npieces = -(-nk // 5)
                    base, rem = divmod(nk, npieces)
                    kk = k0
                    for pi in range(npieces):
                        take = base + (1 if pi < rem else 0)
                        out.append((b, qblk, kk, take, w))
                        kk += take
                else:
                    out.append((b, qblk, k0, nk, w))
            refined.append(out)
        return refined

    best = None
    for tf in [0.70 + 0.02 * i for i in range(21)]:
        target = max(1.0, total_eff / BATCH * tf)
        cores, _ = plan_with(target)
        sc = real_score(refine(cores))
        if best is None or sc < best[0]:
            best = (sc, cores)
    cores = best[1]
    # post-pass: shift whole segments off the slowest core while it helps
    for _ in range(8):
        times = [sum(nk * W_TIME[w] for (_, _, _, nk, w) in segs)
                 + 0.15 * max(0, len(segs) - 1) for segs in cores]
        hi = max(range(BATCH), key=lambda i: times[i])
        lo = min(range(BATCH), key=lambda i: times[i])
        if not cores[hi]:
            break
        seg = min(cores[hi], key=lambda s: s[3] * W_TIME[s[4]])
        move_cost = seg[3] * W_TIME[seg[4]]
        if times[lo] + move_cost + 0.15 < times[hi]:
            cores[hi].remove(seg)
            cores[lo].append(seg)
        else:
            break
    for c in range(BATCH):
        cores[c].sort(key=lambda s: -s[3] * s[4])  # largest segment first
    # refine: split big segments into ~5-unit equal pieces (same block,
    # consecutive ki-ranges) — a mid-kernel store overlaps later compute,
    # so equal thirds beat one monolithic segment by ~0.9us
    refined = []
    for segs in cores:
        out = []
        for (b, qblk, k0, nk, w) in segs:
            if nk >= 9:
                npieces = -(-nk // 5)
                base, rem = divmod(nk, npieces)
                kk = k0
                for pi in range(npieces):
                    take = base + (1 if pi < rem else 0)
                    out.append((b, qblk, kk, take, w))
                    kk += take
            else:
                out.append((b, qblk, k0, nk, w))
        refined.append(out)
    return refined


# ---------------------------------------------------------------------------
# Packed path: per-core program, keyed by the segment-length signature
# ---------------------------------------------------------------------------
_prog_cache = {}


def _build_program(sig):
    """sig: tuple of (nk, w) per segment; w in {512, 1024}."""
    if sig in _prog_cache:
        return _prog_cache[sig]
    nseg = max(1, len(sig))
    units = max(1, sum(nk for (nk, _) in sig))
    qoff = [0]
    for (nk, w) in sig:
        qoff.append(qoff[-1] + w)
    Q = max(512, qoff[-1])
    nc = bass.Bass("TRN2", target_bir_lowering=False, debug=False,
                   num_devices=1)
    qTs = nc.dram_tensor("qTs", [D, Q], f32r, kind="ExternalInput").ap()
    kTs = nc.dram_tensor("kTs", [D, units * 128], f32r, kind="ExternalInput").ap()
    vs = nc.dram_tensor("vs", [128, units, D], bf16, kind="ExternalInput").ap()
    biass = nc.dram_tensor("biass", [128, units], f32, kind="ExternalInput").ap()
    outg = nc.dram_tensor("outg", [D, Q], f32, kind="ExternalOutput").ap()
    deng = nc.dram_tensor("deng", [128, Q], bf16, kind="ExternalOutput").ap()

    with tile.TileContext(nc) as tc:
        with tc.tile_pool(name="const", bufs=1) as const, \
             tc.tile_pool(name="ptp", bufs=6) as ptp, \
             tc.tile_pool(name="denpool", bufs=4) as denpool, \
             tc.tile_pool(name="osb", bufs=3) as osb, \
             tc.tile_pool(name="spsum", bufs=3, space="PSUM") as spsum, \
             tc.tile_pool(name="opsum", bufs=1, space="PSUM") as opsum:

            qTs_sb = const.tile([D, Q], f32r)
            kTs_sb = const.tile([D, units * 128], f32r)
            vs_sb = const.tile([128, units, D], bf16)
            bias_sb = const.tile([128, units], f32)

            def wchunks(w):
                out, p0 = [], 0
                while p0 < w:
                    cw = min(512, w - p0)
                    out.append((p0, cw))
                    p0 += cw
                return out

            if sig:
                # critical path first: ki-chunk 0 (ldweights source), then
                # segment 0's qT block in pieces
                nc.sync.dma_start(out=kTs_sb[:, 0:128], in_=kTs[:, 0:128])
                for (p0, cw) in wchunks(sig[0][1]):
                    nc.sync.dma_start(out=qTs_sb[:, p0:p0 + cw],
                                      in_=qTs[:, p0:p0 + cw])
            nc.gpsimd.dma_start(out=bias_sb[:], in_=biass[:])
            if not sig:
                zf = osb.tile([D, Q], f32, tag="os")
                nc.vector.memset(zf[:], 0.0)
                nc.sync.dma_start(out=outg[:], in_=zf[:])
                zd = denpool.tile([128, Q], bf16, tag="den")
                nc.vector.memset(zd[:], 0.0)
                nc.sync.dma_start(out=deng[:], in_=zd[:])

            units_list = []
            u0 = 0
            seg_u0 = {}
            for i, (nk, w) in enumerate(sig):
                seg_u0[i] = u0
                for u in range(nk):
                    units_list.append((i, u, nk, u0 + u, w))
                u0 += nk

            seg_out_ps = {}
            seg_prev_den = {}

            def load_seg(i, nk, w):
                u_lo = seg_u0[i]
                k_lo = u_lo * 128
                k_hi = (u_lo + nk) * 128
                if i == 0:
                    k_lo += 128     # chunk 0 + qT pre-loaded upfront
                else:
                    nc.sync.dma_start(
                        out=kTs_sb[:, k_lo:k_lo + 128],
                        in_=kTs[:, k_lo:k_lo + 128])
                    k_lo += 128
                    nc.sync.dma_start(
                        out=qTs_sb[:, qoff[i]:qoff[i] + w],
                        in_=qTs[:, qoff[i]:qoff[i] + w])
                if k_hi > k_lo:
                    nc.sync.dma_start(
                        out=kTs_sb[:, k_lo:k_hi], in_=kTs[:, k_lo:k_hi])
                nc.sync.dma_start(
                    out=vs_sb[:, u_lo:u_lo + nk, :],
                    in_=vs[:, u_lo:u_lo + nk, :])

            for i, (nk, w) in enumerate(sig):
                load_seg(i, nk, w)

            def emit_qk(t):
                i, u, nk, col, w = units_list[t]
                s = spsum.tile([128, w], f32, tag="s", name=f"s{t}")
                kch = kTs_sb[:, col * 128:(col + 1) * 128]
                for (p0, cw) in wchunks(w):
                    nc.tensor.matmul(
                        s[:, p0:p0 + cw],
                        kch,
                        qTs_sb[:, qoff[i] + p0: qoff[i] + p0 + cw],
                        start=True, stop=True,
                    )
                return s

            def emit_exp(t, s):
                i, u, nk, col, w = units_list[t]
                pt = ptp.tile([128, w], bf16, tag="pt", name=f"pt{t}")
                halves = 2 if (t == 0 and w == 1024) else 1
                step = w // halves
                for hh in range(halves):
                    sl = slice(hh * step, (hh + 1) * step)
                    nc.scalar.activation(
                        out=pt[:, sl], in_=s[:, sl],
                        func=mybir.ActivationFunctionType.Exp,
                        bias=bias_sb[:, col:col + 1],
                        scale=float(SCALE),
                    )
                return pt

            def emit_av_den(t, pt):
                i, u, nk, col, w = units_list[t]
                if u == 0:
                    seg_out_ps[i] = opsum.tile([D, w], f32, tag="o",
                                               name=f"ops{i}")
                out_ps = seg_out_ps[i]
                for (p0, cw) in wchunks(w):
                    nc.tensor.matmul(
                        out_ps[:, p0:p0 + cw],
                        vs_sb[:, col, :],
                        pt[:, p0:p0 + cw],
                        start=(u == 0), stop=(u == nk - 1),
                        skip_group_check=True,
                    )
                dnew = denpool.tile([128, w], bf16, tag="den", name=f"den{t}")
                if u == 0:
                    nc.vector.tensor_copy(dnew[:], pt[:])
                else:
                    nc.vector.tensor_add(dnew[:], seg_prev_den[i][:], pt[:])
                seg_prev_den[i] = dnew
                if u == nk - 1:
                    last_seg = (i == len(sig) - 1)
                    out_sb = osb.tile([D, w], f32, tag="os", name=f"osb{i}")
                    for hh, (p0, cw) in enumerate(wchunks(w)):
                        sl = slice(p0, p0 + cw)
                        gl = slice(qoff[i] + p0, qoff[i] + p0 + cw)
                        if last_seg:
                            nc.scalar.activation(
                                out=out_sb[:, sl], in_=out_ps[:, sl],
                                func=mybir.ActivationFunctionType.Copy)
                            eng = nc.scalar if hh else nc.sync
                            eng.dma_start(out=outg[:, gl], in_=out_sb[:, sl])
                        else:
                            nc.vector.tensor_copy(out_sb[:, sl], out_ps[:, sl])
                            nc.sync.dma_start(out=outg[:, gl], in_=out_sb[:, sl])
                    nc.sync.dma_start(
                        out=deng[:, qoff[i]:qoff[i] + w], in_=dnew[:])

            # software pipeline: QK of unit t+1 issues before AV of unit t so
            # the in-order PE queue never stalls behind exp_t on ACT
            T = len(units_list)
            if T:
                s_tiles = {t: emit_qk(t) for t in range(min(2, T))}
                for t in range(T):
                    pt = emit_exp(t, s_tiles.pop(t))
                    if t + 2 < T:
                        s_tiles[t + 2] = emit_qk(t + 2)
                    elif t + 2 == T and T > 1:
                        pass
                    emit_av_den(t, pt)

    _split_multi_waits(nc)
    _prog_cache[sig] = nc
    return nc


# ---------------------------------------------------------------------------
# Packed path: execution (one program per core, concurrent dispatch)
# ---------------------------------------------------------------------------
_fn_cache = {}


def _build_callable(nc):
    bass2jax.install_neuronx_cc_hook()
    in_names, out_names, out_avals, zero_outs = [], [], [], []
    for alloc in nc.m.functions[0].allocations:
        if not isinstance(alloc, mybir.MemoryLocationSet):
            continue
        name = alloc.memorylocations[0].name
        if alloc.kind == "ExternalInput":
            in_names.append(name)
        elif alloc.kind == "ExternalOutput":
            shape = tuple(alloc.tensor_shape)
            dtype = mybir.dt.np(alloc.dtype)
            out_names.append(name)
            out_avals.append(jax.core.ShapedArray(shape, dtype))
            zero_outs.append(np.zeros(shape, dtype))
    all_names = in_names + out_names

    def _body(*args):
        outs = bass2jax._bass_exec_p.bind(
            *args,
            out_avals=tuple(out_avals),
            in_names=tuple(all_names),
            out_names=tuple(out_names),
            lowering_input_output_aliases=(),
            sim_require_finite=True,
            sim_require_nnan=True,
            nc=nc,
        )
        return tuple(outs)

    fn = jax.jit(_body, keep_unused=True)
    return fn, in_names, out_names, zero_outs


def _core_inputs(q, k, v, lens, segs):
    units = max(1, sum(nk for (_, _, _, nk, _) in segs))
    Q = max(512, sum(w for (_, _, _, _, w) in segs))
    qTs = np.zeros((D, Q), np.float32)
    kTs = np.zeros((D, units * 128), np.float32)
    vsd = np.zeros((128, units, D), np.float32)
    biass = np.full((128, units), NEG, np.float32)
    u0 = 0
    qo = 0
    p = np.arange(128)
    for (b, qblk, k0, nk, w) in segs:
        L = int(lens[b])
        qTs[:, qo:qo + w] = q[b].T[:, qblk * BQ:qblk * BQ + w]
        for u in range(nk):
            kk = k0 + u
            cc = u0 + u
            kTs[:, cc * 128:(cc + 1) * 128] = k[b].T[:, kk * 128:(kk + 1) * 128]
            vsd[:, cc, :] = v[b][kk * 128:(kk + 1) * 128, :]
            biass[:, cc] = np.where(kk * 128 + p < L, 0.0, NEG)
        u0 += nk
        qo += w
    return {"qTs": qTs, "kTs": kTs, "vs": vsd.astype(_BF16), "biass": biass,
            "partition_id": np.zeros((1, 1), np.uint32)}


def _run_packed(q, k, v, lens):
    cores = _plan(lens)
    sigs = [tuple((nk, w) for (_, _, _, nk, w) in segs) for segs in cores]

    def prep(c):
        nc = _build_program(sigs[c])
        if sigs[c] not in _fn_cache:
            _fn_cache[sigs[c]] = _build_callable(nc)
        return _fn_cache[sigs[c]]

    with ThreadPoolExecutor(max_workers=8) as ex:
        fns = list(ex.map(prep, range(BATCH)))

    devices = jax.devices()[:BATCH]
    results = {}
    for attempt in range(3):
        try:
            futures = []
            for c in range(BATCH):
                fn, in_names, out_names, zero_outs = fns[c]
                in_map = _core_inputs(q, k, v, lens, cores[c])
                args = [jax.device_put(in_map[n], devices[c]) for n in in_names]
                args += [jax.device_put(z, devices[c]) for z in zero_outs]
                futures.append((c, fn(*args), out_names))
            for c, outs, out_names in futures:
                jax.block_until_ready(outs)
                results[c] = {n: np.asarray(outs[i])
                              for i, n in enumerate(out_names)}
            break
        except Exception:
            if attempt == 2:
                raise
            results = {}

    out_acc = np.zeros((BATCH, D, SEQ), np.float64)
    den_acc = np.zeros((BATCH, SEQ), np.float64)
    for c in range(BATCH):
        outg = results[c]["outg"].astype(np.float64)
        deng = results[c]["deng"].astype(np.float64)
        qo = 0
        for (b, qblk, k0, nk, w) in cores[c]:
            sl = slice(qblk * BQ, qblk * BQ + w)
            out_acc[b][:, sl] += outg[:, qo:qo + w]
            den_acc[b][sl] += deng[:, qo:qo + w].sum(axis=0)
            qo += w
    return out_acc, den_acc


# ---------------------------------------------------------------------------
# Dense SPMD fallback (batch b -> core b, full 2048x2048 masked attention)
# ---------------------------------------------------------------------------
_dense_cache = {}


def _build_dense():
    if "nc" in _dense_cache:
        return _dense_cache["nc"]
    nc = bass.Bass("TRN2", target_bir_lowering=False, debug=False,
                   num_devices=BATCH)
    qT = nc.dram_tensor("qT", [D, SEQ], f32r, kind="ExternalInput").ap()
    kT = nc.dram_tensor("kT", [D, SEQ], f32r, kind="ExternalInput").ap()
    v = nc.dram_tensor("v", [SEQ, D], f32, kind="ExternalInput").ap()
    biasm = nc.dram_tensor("biasm", [128, NCH], f32, kind="ExternalInput").ap()
    outT = nc.dram_tensor("outT", [D, SEQ], f32, kind="ExternalOutput").ap()
    denp = nc.dram_tensor("denp", [128, SEQ], bf16, kind="ExternalOutput").ap()

    with tile.TileContext(nc) as tc:
        with tc.tile_pool(name="const", bufs=1) as const, \
             tc.tile_pool(name="ptp", bufs=3) as ptp, \
             tc.tile_pool(name="denpool", bufs=2) as denpool, \
             tc.tile_pool(name="osb", bufs=1) as osb, \
             tc.tile_pool(name="spsum", bufs=2, space="PSUM") as spsum, \
             tc.tile_pool(name="opsum", bufs=1, space="PSUM") as opsum:

            qT_sb = const.tile([D, SEQ], f32r)
            kT_sb = const.tile([D, SEQ], f32r)
            v_sb = const.tile([128, NCH, D], f32)
            v_bf = const.tile([128, NCH, D], bf16)
            bias_sb = const.tile([128, NCH], f32)

            nc.sync.dma_start(out=qT_sb[:], in_=qT[:])
            nc.sync.dma_start(out=kT_sb[:], in_=kT[:])
            nc.sync.dma_start(
                out=v_sb[:], in_=v.rearrange("(c p) d -> p c d", p=128))
            nc.sync.dma_start(out=bias_sb[:], in_=biasm[:])
            nc.vector.tensor_copy(v_bf[:], v_sb[:])

            out_ps = opsum.tile([D, SEQ], f32, tag="ops")
            prev_den = None
            for j in range(NCH):
                kchunk = kT_sb[:, j * 128:(j + 1) * 128]
                pt = ptp.tile([128, SEQ], bf16, tag="pt", name=f"pt{j}")
                for h in range(2):
                    s = spsum.tile([128, SEQ // 2], f32, tag="s",
                                   name=f"s{j}_{h}")
                    for b in range(2):
                        q0 = h * 1024 + b * 512
                        nc.tensor.matmul(
                            s[:, b * 512:(b + 1) * 512],
                            kchunk,
                            qT_sb[:, q0:q0 + 512],
                            start=True, stop=True,
                        )
                    nc.scalar.activation(
                        out=pt[:, h * 1024:(h + 1) * 1024],
                        in_=s[:],
                        func=mybir.ActivationFunctionType.Exp,
                        bias=bias_sb[:, j:j + 1],
                        scale=float(SCALE),
                    )
                for b in range(4):
                    nc.tensor.matmul(
                        out_ps[:, b * 512:(b + 1) * 512],
                        v_bf[:, j, :],
                        pt[:, b * 512:(b + 1) * 512],
                        start=(j == 0), stop=(j == NCH - 1),
                        skip_group_check=True,
                    )
                dnew = denpool.tile([128, SEQ], bf16, tag="den", name=f"dn{j}")
                if prev_den is None:
                    nc.vector.tensor_copy(dnew[:], pt[:])
                else:
                    nc.vector.tensor_add(dnew[:], prev_den[:], pt[:])
                prev_den = dnew

            out_sb = osb.tile([D, SEQ], f32)
            nc.vector.tensor_copy(out_sb[:], out_ps[:])
            nc.sync.dma_start(out=outT[:], in_=out_sb[:])
            nc.sync.dma_start(out=denp[:], in_=prev_den[:])

    _split_multi_waits(nc)
    _dense_cache["nc"] = nc
    return nc


def _run_dense(q, k, v, lens):
    nc = _build_dense()
    col = np.arange(SEQ)
    in_maps = []
    for c in range(BATCH):
        L = int(lens[c])
        biasvec = np.where(col < L, 0.0, NEG).astype(np.float32)
        in_maps.append({
            "qT": np.ascontiguousarray(q[c].T),
            "kT": np.ascontiguousarray(k[c].T),
            "v": np.ascontiguousarray(v[c]),
            "biasm": np.ascontiguousarray(biasvec.reshape(NCH, 128).T),
        })
    last = None
    for attempt in range(3):
        try:
            res = run_bass_kernel_spmd(nc, in_maps, list(range(BATCH)))
            break
        except Exception as e:
            last = e
            if attempt == 2:
                raise last
    out_acc = np.zeros((BATCH, D, SEQ), np.float64)
    den_acc = np.zeros((BATCH, SEQ), np.float64)
    for c in range(BATCH):
        out_acc[c] = res.results[c]["outT"].astype(np.float64)
        den_acc[c] = res.results[c]["denp"].astype(np.float64).sum(axis=0)
    return out_acc, den_acc


# ---------------------------------------------------------------------------
# Entry point
# ---------------------------------------------------------------------------
def kernel(q, k, v, event_lengths):
    q = np.asarray(q, dtype=np.float32)
    k = np.asarray(k, dtype=np.float32)
    v = np.asarray(v, dtype=np.float32)
    lens = np.asarray(event_lengths).astype(np.int64)

    try:
        out_acc, den_acc = _run_packed(q, k, v, lens)
    except Exception:
        out_acc, den_acc = _run_dense(q, k, v, lens)

    out = np.empty((BATCH, SEQ, D), dtype=np.float32)
    col = np.arange(SEQ)
    for b in range(BATCH):
        with np.errstate(divide="ignore", invalid="ignore"):
            o = (out_acc[b] / den_acc[b][None, :]).T
        mean_v = v[b].mean(axis=0)
        valid = (col < int(lens[b]))[:, None]
        out[b] = np.where(valid, o, mean_v[None, :]).astype(np.float32)
    return out

